# revision 2
# baseline (speedup 1.0000x reference)
"""Trainium2 Bass kernel for nn_EdgeDecoder_lgcn (gnn_message_passing).

Computation (reference):
    logit = tanh(z_src @ W1 + b1) @ w2            # [NS]
    beta  = softmax(where(mask, logit, -inf), 1)  # [G, NS]
    agg   = beta @ z_src                          # [G, H]
    scores= agg @ z_dst.T                         # [G, ND]

Two collective-free passes (the baseline's single NEFF carried an
AllReduce, so every core's NEFF sat at the barrier while the other
cores' inputs were still being staged serially — the collective wait
dominated HW exec time by ~4 orders of magnitude):

  Pass 1 (NS sharded): each core computes logits for its z_src slice
  and the partial masked-exp sums U_part = w.T @ [z|1] with
  w[i,g] = mask[g,i]*exp(logit[i]).  Output is only [G, H+1] = 66 KB.
  Pass 2 (ND sharded): host sums the 8 U_parts (tiny), forms
  aggT = (U/s).T, and each core computes scores_slice = aggT.T @ zd.T.

Transfer-size engineering (the axon tunnel moves ~50-75 MB/s, so bytes
on the wire dominate wall clock): z_src/z_dst ship as bf16 (half), the
mask ships bit-packed as uint8 (16x smaller than int32) and is unpacked
on device with shift+and, and scores come back as bf16.

No max-subtraction is needed in the softmax: logit ~ N(0, 0.62), so
exp(logit) is far from overflow and fp32 exp/sums match the reference
closely.
"""

import numpy as np

NS = 50000
ND = 50000
G = 128
H = 128
NCORES = 8
TPD = 49                 # 128-row i-tiles per device
NSL = TPD * 128          # 6272 rows per device slice
NSP = NCORES * NSL       # 50176 padded NS
NB = NSL // 8            # 784 packed mask bytes per group row
NDL = NSL
NDP = NSP
GRP = 4                  # i-tiles batched per 512-wide group
NGRP = (TPD + GRP - 1) // GRP

_CACHE = {}


def _build_pass1(num_devices=NCORES):
    import concourse.bacc as bacc
    import concourse.mybir as mybir
    import concourse.tile as tile
    from concourse import masks

    fp32 = mybir.dt.float32
    bf16 = mybir.dt.bfloat16
    u8 = mybir.dt.uint8

    nc = bacc.Bacc(
        "TRN2", target_bir_lowering=False, debug=False, num_devices=num_devices
    )

    zs = nc.dram_tensor("zs", [NSL, H], bf16, kind="ExternalInput").ap()
    symp = nc.dram_tensor("symp", [G, NB], u8, kind="ExternalInput").ap()
    W1 = nc.dram_tensor("W1", [H, H], fp32, kind="ExternalInput").ap()
    b1 = nc.dram_tensor("b1", [H, 1], fp32, kind="ExternalInput").ap()
    w2 = nc.dram_tensor("w2", [H, 1], fp32, kind="ExternalInput").ap()
    out = nc.dram_tensor("U", [G, H + 1], fp32, kind="ExternalOutput").ap()

    Tanh = mybir.ActivationFunctionType.Tanh
    Exp = mybir.ActivationFunctionType.Exp

    with tile.TileContext(nc) as tc:
        with (
            tc.tile_pool(name="const", bufs=1) as cpool,
            tc.tile_pool(name="big", bufs=1) as big,
            tc.tile_pool(name="sbA", bufs=4) as sbA,
            tc.tile_pool(name="sbB", bufs=4) as sbB,
            tc.tile_pool(name="zt_ps", bufs=2, space="PSUM") as ztp,
            tc.tile_pool(name="t_ps", bufs=2, space="PSUM") as ttp,
            tc.tile_pool(name="mt_ps", bufs=3, space="PSUM") as mtp,
            tc.tile_pool(name="u_ps", bufs=1, space="PSUM") as upl,
        ):
            # ---- constants ----
            ident_bf = cpool.tile([128, 128], bf16)
            masks.make_identity(nc, ident_bf[:])
            W1_sb = cpool.tile([H, H], fp32)          # [h, h'] natural
            nc.sync.dma_start(out=W1_sb[:], in_=W1)
            W1b_sb = cpool.tile([H, H], bf16)
            nc.scalar.copy(W1b_sb[:], W1_sb[:])
            b1_sb = cpool.tile([H, 1], fp32)
            nc.sync.dma_start(out=b1_sb[:], in_=b1)
            w2_sb = cpool.tile([H, 1], fp32)
            nc.sync.dma_start(out=w2_sb[:], in_=w2)
            ones_sb = cpool.tile([H, 1], fp32)
            nc.vector.memset(ones_sb[:], 1.0)

            # ---- bulk inputs (chunked so compute can start early) ----
            # Zs1: partition p holds rows i = 49p + c, c in [0,49), each row
            # followed by a literal 1.0 -> tile c is [:, 129c : 129c+129]
            # = [z_i | 1], giving U and s from one matmul.
            Zs1_sb = big.tile([128, TPD * 129], bf16)
            Zs1v = Zs1_sb[:].rearrange("p (n x) -> p n x", x=129)
            zsv = zs.rearrange("(p n) h -> p n h", p=128)
            bounds = [0, 4, 10, 17, 25, 33, 41, TPD]
            for k in range(len(bounds) - 1):
                lo, hi = bounds[k], bounds[k + 1]
                nc.sync.dma_start(
                    out=Zs1v[:, lo:hi, 0:128], in_=zsv[:, lo:hi, :]
                )
                nc.any.memset(Zs1v[:, lo:hi, 128:129], 1.0)

            # mask: DMA packed bytes, unpack bit b of byte j -> col 8j+b
            pk_sb = cpool.tile([G, NB], u8)
            nc.sync.dma_start(out=pk_sb[:], in_=symp)
            mu_sb = big.tile([G, NSL], u8)
            muv = mu_sb[:].rearrange("g (j b) -> g j b", b=8)
            for b in range(8):
                nc.vector.tensor_scalar(
                    muv[:, :, b], pk_sb[:], 7 - b, 1,
                    mybir.AluOpType.logical_shift_right,
                    mybir.AluOpType.bitwise_and,
                )
            Ms_sb = big.tile([G, NSL], bf16)
            nc.gpsimd.tensor_copy(Ms_sb[:], mu_sb[:])
            # mask col i = 49j + c  ->  [g, j, c] view, c innermost
            Msv = Ms_sb[:].rearrange("g (j c) -> g j c", c=TPD)

            e_sb = cpool.tile([128, TPD], fp32)

            # ---- pass A (logits) interleaved with pass B (U/s accum) ----
            U_ps = upl.tile([G, H + 1], fp32)
            for g in range(NGRP):
                tiles = list(range(g * GRP, min((g + 1) * GRP, TPD)))
                n_t = len(tiles)
                W = n_t * 128
                c0 = tiles[0]
                zT_ps = ztp.tile([128, GRP * 128], bf16, tag="zt")
                for j, c in enumerate(tiles):
                    nc.tensor.transpose(
                        zT_ps[:, j * 128 : (j + 1) * 128],
                        Zs1_sb[:, c * 129 : c * 129 + 128],
                        ident_bf[:],
                    )
                zT_sb = sbA.tile([128, GRP * 128], bf16, tag="zts")
                nc.any.tensor_copy(zT_sb[:, :W], zT_ps[:, :W])
                t_ps = ttp.tile([128, GRP * 128], fp32, tag="tps")
                nc.tensor.matmul(
                    t_ps[:, :W], W1b_sb[:], zT_sb[:, :W], start=True, stop=True
                )
                tanh_sb = sbA.tile([128, GRP * 128], fp32, tag="tanh")
                nc.scalar.activation(
                    tanh_sb[:, :W], t_ps[:, :W], Tanh, bias=b1_sb[:], scale=1.0
                )
                q_sb = sbA.tile([128, GRP * 128], fp32, tag="q")
                nc.vector.tensor_scalar_mul(q_sb[:, :W], tanh_sb[:, :W], w2_sb[:])
                lg_ps = mtp.tile([128, GRP], fp32, tag="mt")
                for j, c in enumerate(tiles):
                    nc.tensor.matmul(
                        lg_ps[:, j : j + 1],
                        q_sb[:, j * 128 : (j + 1) * 128],
                        ones_sb[:],
                        start=True,
                        stop=True,
                    )
                nc.scalar.activation(e_sb[:, c0 : c0 + n_t], lg_ps[:, :n_t], Exp)

                # pass B for this group's tiles: maskT, w = maskT*e, U +=
                mT_ps = mtp.tile([128, GRP * 128], bf16, tag="mt")
                for j, c in enumerate(tiles):
                    nc.tensor.transpose(
                        mT_ps[:, j * 128 : (j + 1) * 128],
                        Msv[:, :, c],
                        ident_bf[:],
                    )
                w_sb = sbB.tile([128, GRP * 128], bf16, tag="w")
                nc.vector.tensor_mul(
                    w_sb[:, :W].rearrange("p (c i) -> p c i", i=128),
                    mT_ps[:, :W].rearrange("p (c i) -> p c i", i=128),
                    e_sb[:, c0 : c0 + n_t].unsqueeze(2).to_broadcast(
                        [128, n_t, 128]
                    ),
                )
                for j, c in enumerate(tiles):
                    nc.tensor.matmul(
                        U_ps[:],
                        w_sb[:, j * 128 : (j + 1) * 128],
                        Zs1_sb[:, c * 129 : (c + 1) * 129],
                        start=(c == 0),
                        stop=(c == TPD - 1),
                    )

            U_sb = sbB.tile([G, H + 1], fp32, tag="uo")
            nc.any.tensor_copy(U_sb[:], U_ps[:])
            nc.sync.dma_start(out=out, in_=U_sb[:])

    nc.compile()
    return nc


def _build_pass2(num_devices=NCORES):
    import concourse.bacc as bacc
    import concourse.mybir as mybir
    import concourse.tile as tile
    from concourse import masks

    fp32 = mybir.dt.float32
    bf16 = mybir.dt.bfloat16

    nc = bacc.Bacc(
        "TRN2", target_bir_lowering=False, debug=False, num_devices=num_devices
    )

    zd = nc.dram_tensor("zd", [NDL, H], bf16, kind="ExternalInput").ap()
    aggT = nc.dram_tensor("aggT", [H, G], fp32, kind="ExternalInput").ap()
    out = nc.dram_tensor("sc", [G, NDL], bf16, kind="ExternalOutput").ap()

    with tile.TileContext(nc) as tc:
        with (
            tc.tile_pool(name="const", bufs=1) as cpool,
            tc.tile_pool(name="big", bufs=1) as big,
            tc.tile_pool(name="sbA", bufs=4) as sbA,
            tc.tile_pool(name="sbD", bufs=4) as sbD,
            tc.tile_pool(name="dt_ps", bufs=2, space="PSUM") as dtp,
            tc.tile_pool(name="d_ps", bufs=4, space="PSUM") as dps,
        ):
            ident_bf = cpool.tile([128, 128], bf16)
            masks.make_identity(nc, ident_bf[:])
            aggT_sb = cpool.tile([H, G], fp32)
            nc.sync.dma_start(out=aggT_sb[:], in_=aggT)

            # tile n = zd rows [128n, 128n+128), partition = row offset, so
            # transposed tiles cover contiguous output column blocks
            Zd_sb = big.tile([128, TPD * 128], bf16)
            Zdv = Zd_sb[:].rearrange("p (n h) -> p n h", h=H)
            zdv = zd.rearrange("(n p) h -> p n h", p=128)
            bounds = [0, 4, 10, 17, 25, 33, 41, TPD]
            for k in range(len(bounds) - 1):
                lo, hi = bounds[k], bounds[k + 1]
                nc.sync.dma_start(out=Zdv[:, lo:hi, :], in_=zdv[:, lo:hi, :])

            for m in range(NGRP):
                tiles = list(range(m * GRP, min((m + 1) * GRP, TPD)))
                n_t = len(tiles)
                W = n_t * 128
                lo = tiles[0] * 128
                dT_ps = dtp.tile([128, GRP * 128], bf16, tag="dt")
                for j, n in enumerate(tiles):
                    nc.tensor.transpose(
                        dT_ps[:, j * 128 : (j + 1) * 128],
                        Zdv[:, n, :],
                        ident_bf[:],
                    )
                dT_sb = sbA.tile([128, GRP * 128], fp32, tag="dts")
                nc.any.tensor_copy(dT_sb[:, :W], dT_ps[:, :W])
                sc_ps = dps.tile([G, GRP * 128], fp32, tag="sc")
                nc.tensor.matmul(
                    sc_ps[:, :W], aggT_sb[:], dT_sb[:, :W], start=True, stop=True
                )
                o_sb = sbD.tile([G, GRP * 128], bf16, tag="o")
                nc.any.tensor_copy(o_sb[:, :W], sc_ps[:, :W])
                eng = nc.sync if m % 2 == 0 else nc.scalar
                eng.dma_start(out=out[:, lo : lo + W], in_=o_sb[:, :W])

    nc.compile()
    return nc


def _get_modules():
    if "nc1" not in _CACHE:
        _CACHE["nc1"] = _build_pass1()
        _CACHE["nc2"] = _build_pass2()
    return _CACHE["nc1"], _CACHE["nc2"]


def make_in_maps1(z_src, sym_indexs, W1, b1, w2):
    import ml_dtypes

    bf16 = ml_dtypes.bfloat16
    W1 = np.ascontiguousarray(np.asarray(W1, dtype=np.float32))
    b1 = np.ascontiguousarray(np.asarray(b1, dtype=np.float32)).reshape(H, 1)
    w2 = np.ascontiguousarray(np.asarray(w2, dtype=np.float32)).reshape(H, 1)

    zsp = np.zeros((NSP, H), dtype=bf16)
    zsp[:NS] = np.asarray(z_src, dtype=np.float32).astype(bf16)
    symb = np.zeros((G, NSP), dtype=np.uint8)
    symb[:, :NS] = np.asarray(sym_indexs).astype(np.uint8)
    symp = np.packbits(symb, axis=1)  # [G, NSP/8]

    in_maps = []
    for k in range(NCORES):
        lo = k * NSL
        in_maps.append(
            {
                "zs": np.ascontiguousarray(zsp[lo : lo + NSL]),
                "symp": np.ascontiguousarray(symp[:, k * NB : (k + 1) * NB]),
                "W1": W1,
                "b1": b1,
                "w2": w2,
            }
        )
    return in_maps


def make_in_maps2(z_dst, aggT):
    import ml_dtypes

    bf16 = ml_dtypes.bfloat16
    zdp = np.zeros((NDP, H), dtype=bf16)
    zdp[:ND] = np.asarray(z_dst, dtype=np.float32).astype(bf16)
    in_maps = []
    for k in range(NCORES):
        lo = k * NDL
        in_maps.append(
            {"zd": np.ascontiguousarray(zdp[lo : lo + NDL]), "aggT": aggT}
        )
    return in_maps


def combine_U(res1):
    U = np.zeros((G, H + 1), dtype=np.float32)
    for k in range(NCORES):
        U += np.asarray(res1.results[k]["U"], dtype=np.float32)
    aggT = np.ascontiguousarray((U[:, :H] / U[:, H : H + 1]).T)
    return aggT


def kernel(z_src, z_dst, sym_indexs, W1, b1, w2):
    from concourse import bass_utils

    nc1, nc2 = _get_modules()
    in_maps1 = make_in_maps1(z_src, sym_indexs, W1, b1, w2)
    res1 = bass_utils.run_bass_kernel_spmd(
        nc1, in_maps1, core_ids=list(range(NCORES))
    )
    aggT = combine_U(res1)
    in_maps2 = make_in_maps2(z_dst, aggT)
    res2 = bass_utils.run_bass_kernel_spmd(
        nc2, in_maps2, core_ids=list(range(NCORES))
    )
    scores = np.empty((G, NDP), dtype=np.float32)
    for k in range(NCORES):
        scores[:, k * NDL : (k + 1) * NDL] = res2.results[k]["sc"]
    return scores[:, :ND]


if __name__ == "__main__":
    rng = np.random.default_rng(0)
    inputs = {
        "z_src": rng.standard_normal((NS, H), dtype=np.float32),
        "z_dst": rng.standard_normal((ND, H), dtype=np.float32),
        "sym_indexs": rng.integers(0, 2, (G, NS), dtype=np.int32),
        "W1": rng.standard_normal((H, H), dtype=np.float32) / np.sqrt(H),
        "b1": np.zeros(H, dtype=np.float32),
        "w2": rng.standard_normal(H, dtype=np.float32) / np.sqrt(H),
    }
    out = kernel(**inputs)
    print(out.shape, out.dtype, np.abs(out).max())
